# revision 3
# baseline (speedup 1.0000x reference)
"""Trainium2 Bass kernel for a single-head causal attention block.

Problem (per batch element b, B=8 sharded 1:1 onto 8 NeuronCores):
    q = x @ Wq.T + bq ; k = x @ Wk.T + bk ; v = x @ Wv.T + bv      [S, H]
    scores = q @ k.T / sqrt(H), causal mask, softmax over t
    out = (softmax @ v) @ Wo.T + bo                                 [S, H]

Device-side dataflow (all big matmuls in bf16, 1 cycle/row on PE):
    - host ships x[b].T (bf16) so the contraction dim d sits on partitions
    - one stacked projection pass computes [Q^T; K^T] (m=128), one pass V^T
    - K^T is moved from partitions 64..127 to 0..63 with a selector matmul
    - V^T tiles are transposed on the PE (128x64) and get a ones-column ->
      the PV matmul accumulates unnormalized output AND the softmax row-sum
    - scores are computed transposed (ST[t, s]) so exp(ST) feeds the PV
      matmul directly as the moving operand; causal masking is an
      affine_select (zero-fill) on the GPSIMD engine, diagonal tiles only
    - final projection uses rhs [[Wo^T, 0], [bo, 1]] so dividing by the
      carried row-sum also applies bo correctly (fp32 matmul, tiny)
"""

import numpy as np
import ml_dtypes

import concourse.bass as bass
import concourse.bacc as bacc
import concourse.mybir as mybir
from concourse.tile import TileContext
from concourse.bass_utils import run_bass_kernel_spmd

B, S, D, H = 8, 2048, 1024, 64
NCORES = 8
PT = 128           # partition tile
SB = 512           # s-block width (moving-operand width)
NSB = S // SB      # 4
NDT = D // PT      # 8 d-tiles
TPB = SB // PT     # 4 t-tiles per s-block
BF16 = mybir.dt.bfloat16
F32 = mybir.dt.float32
BF16NP = ml_dtypes.bfloat16

_NC_CACHE = {}


def _build():
    nc = bacc.Bacc()
    xt_d = nc.dram_tensor("xt", [D, S], BF16, kind="ExternalInput")
    wqk_d = nc.dram_tensor("wqk", [D, PT], BF16, kind="ExternalInput")
    wv_d = nc.dram_tensor("wv", [D, H], BF16, kind="ExternalInput")
    e1_d = nc.dram_tensor("e1", [PT, H], BF16, kind="ExternalInput")
    idn_d = nc.dram_tensor("idn", [H, H], BF16, kind="ExternalInput")
    wo_d = nc.dram_tensor("wo", [H + 1, H + 1], F32, kind="ExternalInput")
    bqk_d = nc.dram_tensor("bqk", [PT, 1], F32, kind="ExternalInput")
    bv_d = nc.dram_tensor("bv", [H, 1], F32, kind="ExternalInput")
    y_d = nc.dram_tensor("y", [S, H], F32, kind="ExternalOutput")

    scale = 1.0 / float(np.sqrt(H))

    with TileContext(nc) as tc:
        with (
            tc.tile_pool(name="consts", bufs=1) as consts,
            tc.tile_pool(name="persist", bufs=1) as persist,
            tc.tile_pool(name="work", bufs=3) as work,
            tc.tile_pool(name="expp", bufs=3) as expp,
            tc.tile_pool(name="otp", bufs=2) as otp,
            tc.tile_pool(name="yp", bufs=2) as yp,
            tc.tile_pool(name="small", bufs=4) as small,
            tc.tile_pool(name="ps", bufs=4, space="PSUM") as ps,
            tc.tile_pool(name="ps_st", bufs=2, space="PSUM") as ps_st,
        ):
            # ---- load constants / x^T ----
            xt_sb = persist.tile([PT, NDT, S], BF16, tag="xt")
            for k in range(NDT):
                nc.sync.dma_start(out=xt_sb[:, k, :], in_=xt_d[k * PT:(k + 1) * PT, :])
            wqk_sb = consts.tile([PT, NDT, PT], BF16, tag="wqk")
            nc.sync.dma_start(out=wqk_sb, in_=wqk_d[:].rearrange("(k p) m -> p k m", p=PT))
            wv_sb = consts.tile([PT, NDT, H], BF16, tag="wv")
            nc.sync.dma_start(out=wv_sb, in_=wv_d[:].rearrange("(k p) m -> p k m", p=PT))
            e1_sb = consts.tile([PT, H], BF16, tag="e1")
            nc.sync.dma_start(out=e1_sb, in_=e1_d[:])
            idn_sb = consts.tile([H, H], BF16, tag="idn")
            nc.sync.dma_start(out=idn_sb, in_=idn_d[:])
            wo_sb = consts.tile([H + 1, H + 1], F32, tag="wo")
            nc.sync.dma_start(out=wo_sb, in_=wo_d[:])
            bqk_sb = consts.tile([PT, 1], F32, tag="bqk")
            nc.sync.dma_start(out=bqk_sb, in_=bqk_d[:])
            bv_sb = consts.tile([H, 1], F32, tag="bv")
            nc.sync.dma_start(out=bv_sb, in_=bv_d[:])

            # warm the ACT exp table while DMA streams in
            warm_in = small.tile([PT, 1], F32, tag="warm_in")
            nc.vector.memset(warm_in, 0.0)
            warm_out = small.tile([PT, 1], F32, tag="warm_out")
            nc.scalar.activation(warm_out, warm_in, mybir.ActivationFunctionType.Exp)

            qkt = []   # per s-block [128, SB] bf16: rows 0:64 Q^T, rows 64:128 K^T
            kt = []    # per s-block [64, SB] bf16: K^T moved to partitions 0..63
            vext = []  # per t-tile [128, H+1] bf16: [V | 1]

            for j in range(NSB):
                scol = slice(j * SB, (j + 1) * SB)

                # ---- projections for s-block j ----
                ps_qk = ps.tile([PT, SB], F32, tag="ps")
                for k in range(NDT):
                    nc.tensor.matmul(ps_qk, lhsT=wqk_sb[:, k, :],
                                     rhs=xt_sb[:, k, scol],
                                     start=(k == 0), stop=(k == NDT - 1))
                qkt_j = persist.tile([PT, SB], BF16, tag=f"qkt{j}")
                nc.vector.tensor_scalar_add(qkt_j, ps_qk, bqk_sb)
                qkt.append(qkt_j)

                # K^T rows 64..127 -> partitions 0..63 (selector matmul)
                ps_kt = ps.tile([H, SB], F32, tag="ps")
                nc.tensor.matmul(ps_kt, lhsT=e1_sb, rhs=qkt_j, start=True, stop=True)
                kt_j = persist.tile([H, SB], BF16, tag=f"kt{j}")
                nc.vector.tensor_copy(kt_j, ps_kt)
                kt.append(kt_j)

                # V^T for this s-block
                ps_v = ps.tile([H, SB], F32, tag="ps")
                for k in range(NDT):
                    nc.tensor.matmul(ps_v, lhsT=wv_sb[:, k, :],
                                     rhs=xt_sb[:, k, scol],
                                     start=(k == 0), stop=(k == NDT - 1))
                vt_j = work.tile([H, SB], BF16, tag="vt")
                nc.vector.tensor_scalar_add(vt_j, ps_v, bv_sb)

                # V^T -> V tiles [128, 64] on the PE; append ones column
                for u in range(TPB):
                    tt = j * TPB + u
                    ps_vt = ps.tile([PT, H], BF16, tag="ps")
                    nc.tensor.transpose(ps_vt, vt_j[:, u * PT:(u + 1) * PT], idn_sb)
                    vx = persist.tile([PT, H + 1], BF16, tag=f"vext{tt}")
                    nc.vector.tensor_copy(vx[:, 0:H], ps_vt)
                    nc.vector.memset(vx[:, H:H + 1], 1.0)
                    vext.append(vx)

                # ---- attention for s-block j ----
                ps_o = ps.tile([H + 1, SB], F32, tag="ps")
                npair = 2 * (j + 1)
                last_tt = (j + 1) * TPB - 1
                for p in range(npair):
                    pair = (2 * p, 2 * p + 1)
                    st = ps_st.tile([PT, 2 * SB], F32, tag="st")
                    for q, tt in enumerate(pair):
                        jb, u = tt // TPB, tt % TPB
                        nc.tensor.matmul(st[:, q * SB:(q + 1) * SB],
                                         lhsT=kt[jb][:, u * PT:(u + 1) * PT],
                                         rhs=qkt[j][0:H, :],
                                         start=True, stop=True)
                    ex = expp.tile([PT, 2 * SB], BF16, tag="exp")
                    nc.scalar.activation(ex, st, mybir.ActivationFunctionType.Exp,
                                         scale=scale)
                    for q, tt in enumerate(pair):
                        if tt >= j * TPB:  # diagonal tile: zero where t > s
                            nc.gpsimd.affine_select(
                                out=ex[:, q * SB:(q + 1) * SB],
                                in_=ex[:, q * SB:(q + 1) * SB],
                                compare_op=mybir.AluOpType.is_ge,
                                fill=0.0,
                                base=j * SB - tt * PT,
                                pattern=[[1, SB]],
                                channel_multiplier=-1,
                            )
                    for q, tt in enumerate(pair):
                        nc.tensor.matmul(ps_o, lhsT=vext[tt],
                                         rhs=ex[:, q * SB:(q + 1) * SB],
                                         start=(tt == 0), stop=(tt == last_tt))
                ot = otp.tile([H + 1, SB], F32, tag="ot")
                nc.vector.tensor_copy(ot, ps_o)

                # ---- output projection + normalization ----
                ystage = yp.tile([PT, TPB, H], F32, tag="y")
                for u in range(TPB):
                    ps_f = ps.tile([PT, H + 1], F32, tag="ps")
                    nc.tensor.matmul(ps_f, lhsT=ot[:, u * PT:(u + 1) * PT],
                                     rhs=wo_sb, start=True, stop=True)
                    rec = small.tile([PT, 1], F32, tag="rec")
                    nc.vector.reciprocal(rec, ps_f[:, H:H + 1])
                    nc.vector.tensor_scalar_mul(ystage[:, u, :], ps_f[:, 0:H], rec)
                nc.sync.dma_start(
                    out=y_d[j * SB:(j + 1) * SB, :].rearrange("(u p) g -> p u g", p=PT),
                    in_=ystage)
    nc.finalize()  # bacc compile: register alloc + wait splitting (TRN2: <=1 wait/inst)
    return nc


def get_nc():
    if "nc" not in _NC_CACHE:
        _NC_CACHE["nc"] = _build()
    return _NC_CACHE["nc"]


def make_in_maps(x, Wq, bq, Wk, bk, Wv, bv, Wo, bo):
    x = np.asarray(x, np.float32)
    wqk = np.ascontiguousarray(
        np.concatenate([np.asarray(Wq).T, np.asarray(Wk).T], axis=1)).astype(BF16NP)
    wv = np.ascontiguousarray(np.asarray(Wv).T).astype(BF16NP)
    e1 = np.zeros((PT, H), BF16NP)
    e1[H + np.arange(H), np.arange(H)] = 1
    idn = np.eye(H, dtype=BF16NP)
    wo = np.zeros((H + 1, H + 1), np.float32)
    wo[:H, :H] = np.asarray(Wo).T
    wo[H, :H] = np.asarray(bo)
    wo[H, H] = 1.0
    bqk = np.concatenate([np.asarray(bq), np.asarray(bk)])[:, None].astype(np.float32)
    bvv = np.asarray(bv)[:, None].astype(np.float32)
    shared = {"wqk": wqk, "wv": wv, "e1": e1, "idn": idn, "wo": wo,
              "bqk": bqk, "bv": bvv}
    return [dict(shared, xt=np.ascontiguousarray(x[b].T).astype(BF16NP))
            for b in range(B)]


def kernel(**inputs):
    nc = get_nc()
    in_maps = make_in_maps(**inputs)
    res = run_bass_kernel_spmd(nc, in_maps, list(range(NCORES)))
    return np.stack([res.results[i]["y"] for i in range(NCORES)], axis=0)


# revision 4
# speedup vs baseline: 1.2360x; 1.2360x over previous
"""Trainium2 Bass kernel for a single-head causal attention block (v2).

v2 over v1:
  - one swap-selector matmul per s-block produces ktq = [K^T; Q^T] so both
    PE row-groups have aligned operands -> score matmuls run row-PACKED
    (two K=64 matmuls concurrent in the 128x128 array, ~2x ST throughput)
  - V projection is col-packed: two s-blocks' V^T computed concurrently in
    the two column-group halves (m=64 each)
  - x^T DMA split into s-halves so the first projections/exps start after
    ~half the input has landed
"""

import numpy as np
import ml_dtypes

import concourse.bass as bass
import concourse.bacc as bacc
import concourse.mybir as mybir
from concourse.tile import TileContext
from concourse.bass_utils import run_bass_kernel_spmd

B, S, D, H = 8, 2048, 1024, 64
NCORES = 8
PT = 128           # partition tile
SB = 512           # s-block width (moving-operand width)
NSB = S // SB      # 4
NDT = D // PT      # 8 d-tiles
TPB = SB // PT     # 4 t-tiles per s-block
BF16 = mybir.dt.bfloat16
F32 = mybir.dt.float32
BF16NP = ml_dtypes.bfloat16

_NC_CACHE = {}


def _build():
    nc = bacc.Bacc()
    xt_d = nc.dram_tensor("xt", [D, S], BF16, kind="ExternalInput")
    wqk_d = nc.dram_tensor("wqk", [D, PT], BF16, kind="ExternalInput")
    wv_d = nc.dram_tensor("wv", [D, H], BF16, kind="ExternalInput")
    e12_d = nc.dram_tensor("e12", [PT, PT], BF16, kind="ExternalInput")
    idn2_d = nc.dram_tensor("idn2", [PT, H], BF16, kind="ExternalInput")
    wo_d = nc.dram_tensor("wo", [H + 1, H + 1], F32, kind="ExternalInput")
    bqk_d = nc.dram_tensor("bqk", [PT, 1], F32, kind="ExternalInput")
    bv2_d = nc.dram_tensor("bv2", [PT, 1], F32, kind="ExternalInput")
    y_d = nc.dram_tensor("y", [S, H], F32, kind="ExternalOutput")

    scale = 1.0 / float(np.sqrt(H))

    with TileContext(nc) as tc:
        with (
            tc.tile_pool(name="consts", bufs=1) as consts,
            tc.tile_pool(name="persist", bufs=1) as persist,
            tc.tile_pool(name="work", bufs=2) as work,
            tc.tile_pool(name="expp", bufs=3) as expp,
            tc.tile_pool(name="otp", bufs=2) as otp,
            tc.tile_pool(name="yp", bufs=2) as yp,
            tc.tile_pool(name="small", bufs=4) as small,
            tc.tile_pool(name="ps", bufs=4, space="PSUM") as ps,
            tc.tile_pool(name="ps_st", bufs=2, space="PSUM") as ps_st,
        ):
            # ---- tiny consts FIRST so the PE can start as soon as x arrives
            wqk_sb = consts.tile([PT, NDT, PT], BF16, tag="wqk")
            nc.sync.dma_start(out=wqk_sb, in_=wqk_d[:].rearrange("(k p) m -> p k m", p=PT))
            wv_sb = consts.tile([PT, NDT, H], BF16, tag="wv")
            nc.sync.dma_start(out=wv_sb, in_=wv_d[:].rearrange("(k p) m -> p k m", p=PT))
            e12_sb = consts.tile([PT, PT], BF16, tag="e12")
            nc.sync.dma_start(out=e12_sb, in_=e12_d[:])
            idn2_sb = consts.tile([PT, H], BF16, tag="idn2")
            nc.sync.dma_start(out=idn2_sb, in_=idn2_d[:])
            wo_sb = consts.tile([H + 1, H + 1], F32, tag="wo")
            nc.sync.dma_start(out=wo_sb, in_=wo_d[:])
            bqk_sb = consts.tile([PT, 1], F32, tag="bqk")
            nc.sync.dma_start(out=bqk_sb, in_=bqk_d[:])
            bv2_sb = consts.tile([PT, 1], F32, tag="bv2")
            nc.sync.dma_start(out=bv2_sb, in_=bv2_d[:])

            # ---- x^T: split each d-tile DMA into s-halves; first halves first
            xt_sb = persist.tile([PT, NDT, S], BF16, tag="xt")
            for half in range(2):
                hcol = slice(half * (S // 2), (half + 1) * (S // 2))
                for k in range(NDT):
                    nc.sync.dma_start(out=xt_sb[:, k, hcol],
                                      in_=xt_d[k * PT:(k + 1) * PT, hcol])

            # warm the ACT exp table while DMA streams in
            warm_in = small.tile([PT, 1], F32, tag="warm_in")
            nc.vector.memset(warm_in, 0.0)
            warm_out = small.tile([PT, 1], F32, tag="warm_out")
            nc.scalar.activation(warm_out, warm_in, mybir.ActivationFunctionType.Exp)

            qkt = [None] * NSB   # [128, SB] bf16: rows 0:64 Q^T, 64:128 K^T
            ktq = [None] * NSB   # [128, SB] bf16: rows 0:64 K^T, 64:128 Q^T
            vext = [None] * (NSB * TPB)  # [128, H+1] bf16: [V | 1]

            def proj_qk(j):
                scol = slice(j * SB, (j + 1) * SB)
                ps_qk = ps.tile([PT, SB], F32, tag="ps")
                for k in range(NDT):
                    nc.tensor.matmul(ps_qk, lhsT=wqk_sb[:, k, :],
                                     rhs=xt_sb[:, k, scol],
                                     start=(k == 0), stop=(k == NDT - 1))
                qkt_j = persist.tile([PT, SB], BF16, tag=f"qkt{j}")
                nc.vector.tensor_scalar_add(qkt_j, ps_qk, bqk_sb)
                qkt[j] = qkt_j
                # swap-selector: ktq = [K^T; Q^T]
                ps_sw = ps.tile([PT, SB], F32, tag="ps")
                nc.tensor.matmul(ps_sw, lhsT=e12_sb, rhs=qkt_j, start=True, stop=True)
                ktq_j = persist.tile([PT, SB], BF16, tag=f"ktq{j}")
                nc.vector.tensor_copy(ktq_j, ps_sw)
                ktq[j] = ktq_j

            def proj_v_packed(j0, j1):
                ps_vv = ps.tile([PT, SB], F32, tag="ps")
                c0 = slice(j0 * SB, (j0 + 1) * SB)
                c1 = slice(j1 * SB, (j1 + 1) * SB)
                for k in range(NDT):
                    nc.tensor.matmul(ps_vv[0:H, :], lhsT=wv_sb[:, k, :],
                                     rhs=xt_sb[:, k, c0],
                                     start=(k == 0), stop=(k == NDT - 1),
                                     skip_group_check=True)
                    nc.tensor.matmul(ps_vv[H:PT, :], lhsT=wv_sb[:, k, :],
                                     rhs=xt_sb[:, k, c1],
                                     start=(k == 0), stop=(k == NDT - 1),
                                     skip_group_check=True)
                vt2 = work.tile([PT, SB], BF16, tag="vt")
                nc.vector.tensor_scalar_add(vt2, ps_vv, bv2_sb)
                for jj, base in ((j0, 0), (j1, H)):
                    for u in range(TPB):
                        tt = jj * TPB + u
                        ps_vt = ps.tile([PT, H], BF16, tag="ps")
                        nc.tensor.transpose(ps_vt,
                                            vt2[base:base + H, u * PT:(u + 1) * PT],
                                            idn2_sb[base:base + H, :])
                        vx = persist.tile([PT, H + 1], BF16, tag=f"vext{tt}")
                        nc.vector.tensor_copy(vx[:, 0:H], ps_vt)
                        nc.vector.memset(vx[:, H:H + 1], 1.0)
                        vext[tt] = vx

            def attn(j):
                ps_o = ps.tile([H + 1, SB], F32, tag="ps")
                last_tt = (j + 1) * TPB - 1
                for p in range(2 * (j + 1)):
                    pair = (2 * p, 2 * p + 1)
                    st = ps_st.tile([PT, 2 * SB], F32, tag="st")
                    # row-packed score matmuls: even tile in rows 0-63,
                    # odd tile in rows 64-127 (concurrent in the PE array)
                    jb0, u0 = divmod(pair[0], TPB)
                    nc.tensor.matmul(st[:, 0:SB],
                                     lhsT=ktq[jb0][0:H, u0 * PT:(u0 + 1) * PT],
                                     rhs=qkt[j][0:H, :],
                                     start=True, stop=True)
                    jb1, u1 = divmod(pair[1], TPB)
                    nc.tensor.matmul(st[:, SB:2 * SB],
                                     lhsT=qkt[jb1][H:PT, u1 * PT:(u1 + 1) * PT],
                                     rhs=ktq[j][H:PT, :],
                                     start=True, stop=True)
                    ex = expp.tile([PT, 2 * SB], BF16, tag="exp")
                    nc.scalar.activation(ex, st, mybir.ActivationFunctionType.Exp,
                                         scale=scale)
                    for q, tt in enumerate(pair):
                        if tt >= j * TPB:  # diagonal tile: zero where t > s
                            nc.gpsimd.affine_select(
                                out=ex[:, q * SB:(q + 1) * SB],
                                in_=ex[:, q * SB:(q + 1) * SB],
                                compare_op=mybir.AluOpType.is_ge,
                                fill=0.0,
                                base=j * SB - tt * PT,
                                pattern=[[1, SB]],
                                channel_multiplier=-1,
                            )
                    for q, tt in enumerate(pair):
                        nc.tensor.matmul(ps_o, lhsT=vext[tt],
                                         rhs=ex[:, q * SB:(q + 1) * SB],
                                         start=(tt == 0), stop=(tt == last_tt))
                ot = otp.tile([H + 1, SB], F32, tag="ot")
                nc.vector.tensor_copy(ot, ps_o)

                ystage = yp.tile([PT, TPB, H], F32, tag="y")
                for u in range(TPB):
                    ps_f = ps.tile([PT, H + 1], F32, tag="ps")
                    nc.tensor.matmul(ps_f, lhsT=ot[:, u * PT:(u + 1) * PT],
                                     rhs=wo_sb, start=True, stop=True)
                    rec = small.tile([PT, 1], F32, tag="rec")
                    nc.vector.reciprocal(rec, ps_f[:, H:H + 1])
                    nc.vector.tensor_scalar_mul(ystage[:, u, :], ps_f[:, 0:H], rec)
                nc.sync.dma_start(
                    out=y_d[j * SB:(j + 1) * SB, :].rearrange("(u p) g -> p u g", p=PT),
                    in_=ystage)

            for jp in range(NSB // 2):
                j0, j1 = 2 * jp, 2 * jp + 1
                proj_qk(j0)
                proj_qk(j1)
                proj_v_packed(j0, j1)
                attn(j0)
                attn(j1)
    nc.finalize()  # bacc compile: register alloc + wait splitting (TRN2: <=1 wait/inst)
    return nc


def get_nc():
    if "nc" not in _NC_CACHE:
        _NC_CACHE["nc"] = _build()
    return _NC_CACHE["nc"]


def make_in_maps(x, Wq, bq, Wk, bk, Wv, bv, Wo, bo):
    x = np.asarray(x, np.float32)
    wqk = np.ascontiguousarray(
        np.concatenate([np.asarray(Wq).T, np.asarray(Wk).T], axis=1)).astype(BF16NP)
    wv = np.ascontiguousarray(np.asarray(Wv).T).astype(BF16NP)
    e12 = np.zeros((PT, PT), BF16NP)  # swap matrix: out rows = [in 64:128; in 0:64]
    e12[H + np.arange(H), np.arange(H)] = 1
    e12[np.arange(H), H + np.arange(H)] = 1
    idn2 = np.zeros((PT, H), BF16NP)
    idn2[0:H, :] = np.eye(H)
    idn2[H:PT, :] = np.eye(H)
    wo = np.zeros((H + 1, H + 1), np.float32)
    wo[:H, :H] = np.asarray(Wo).T
    wo[H, :H] = np.asarray(bo)
    wo[H, H] = 1.0
    bqk = np.concatenate([np.asarray(bq), np.asarray(bk)])[:, None].astype(np.float32)
    bv2 = np.concatenate([np.asarray(bv), np.asarray(bv)])[:, None].astype(np.float32)
    shared = {"wqk": wqk, "wv": wv, "e12": e12, "idn2": idn2, "wo": wo,
              "bqk": bqk, "bv2": bv2}
    return [dict(shared, xt=np.ascontiguousarray(x[b].T).astype(BF16NP))
            for b in range(B)]


def kernel(**inputs):
    nc = get_nc()
    in_maps = make_in_maps(**inputs)
    res = run_bass_kernel_spmd(nc, in_maps, list(range(NCORES)))
    return np.stack([res.results[i]["y"] for i in range(NCORES)], axis=0)


# revision 5
# speedup vs baseline: 1.2370x; 1.0008x over previous
"""Trainium2 Bass kernel for a single-head causal attention block (v3).

v3 over v2 (from the NTFF trace: 7us engine preamble, ~650ns serialized
issue cost PER dma_start on the sync queue, HAM cold through the DMA-paced
projection phase, ACT starved until late projections finished):
  - all constants packed host-side into TWO dram tensors -> 2 DMA triggers
  - x^T loaded as 8 big [128, 2048] DMAs (fewer triggers)
  - ALL projections emitted first (dense PE work -> HAM warms once),
    then all attention blocks (continuous ACT exp stream from ~20us)
"""

import numpy as np
import ml_dtypes

import concourse.bass as bass
import concourse.bacc as bacc
import concourse.mybir as mybir
from concourse.tile import TileContext
from concourse.bass_utils import run_bass_kernel_spmd

B, S, D, H = 8, 2048, 1024, 64
NCORES = 8
PT = 128           # partition tile
SB = 512           # s-block width (moving-operand width)
NSB = S // SB      # 4
NDT = D // PT      # 8 d-tiles
TPB = SB // PT     # 4 t-tiles per s-block
BF16 = mybir.dt.bfloat16
F32 = mybir.dt.float32
BF16NP = ml_dtypes.bfloat16

_NC_CACHE = {}


def _build():
    nc = bacc.Bacc()
    xt_d = nc.dram_tensor("xt", [D, S], BF16, kind="ExternalInput")
    # packed consts: cst16 = [wqk(8x128) | wv(8x64) | e12(128) | idn2(64)]
    cst16_d = nc.dram_tensor("cst16", [PT, 1728], BF16, kind="ExternalInput")
    # cst32 = [wo(65, zero-padded to 128 rows) | bqk(1) | bv2(1)]
    cst32_d = nc.dram_tensor("cst32", [PT, H + 3], F32, kind="ExternalInput")
    y_d = nc.dram_tensor("y", [S, H], F32, kind="ExternalOutput")

    scale = 1.0 / float(np.sqrt(H))

    with TileContext(nc) as tc:
        with (
            tc.tile_pool(name="consts", bufs=1) as consts,
            tc.tile_pool(name="persist", bufs=1) as persist,
            tc.tile_pool(name="work", bufs=2) as work,
            tc.tile_pool(name="expp", bufs=3) as expp,
            tc.tile_pool(name="otp", bufs=2) as otp,
            tc.tile_pool(name="yp", bufs=2) as yp,
            tc.tile_pool(name="small", bufs=4) as small,
            tc.tile_pool(name="ps", bufs=4, space="PSUM") as ps,
            tc.tile_pool(name="ps_st", bufs=2, space="PSUM") as ps_st,
        ):
            # ---- consts in two packed DMAs, then 8 big x^T DMAs
            cst16 = consts.tile([PT, 1728], BF16, tag="cst16")
            nc.sync.dma_start(out=cst16, in_=cst16_d[:])
            cst32 = consts.tile([PT, H + 3], F32, tag="cst32")
            nc.sync.dma_start(out=cst32, in_=cst32_d[:])
            wqk_sb = cst16[:, 0:NDT * PT].rearrange("p (k m) -> p k m", k=NDT)
            wv_sb = cst16[:, NDT * PT:NDT * PT + NDT * H].rearrange(
                "p (k m) -> p k m", k=NDT)
            e12_sb = cst16[:, 1536:1536 + PT]
            idn2_sb = cst16[:, 1664:1664 + H]
            wo_sb = cst32[0:H + 1, 0:H + 1]
            bqk_sb = cst32[:, H + 1:H + 2]
            bv2_sb = cst32[:, H + 2:H + 3]

            xt_sb = persist.tile([PT, NDT, S], BF16, tag="xt")
            for k in range(NDT):
                nc.sync.dma_start(out=xt_sb[:, k, :],
                                  in_=xt_d[k * PT:(k + 1) * PT, :])

            # warm the ACT exp table while DMA streams in
            warm_in = small.tile([PT, 1], F32, tag="warm_in")
            nc.vector.memset(warm_in, 0.0)
            warm_out = small.tile([PT, 1], F32, tag="warm_out")
            nc.scalar.activation(warm_out, warm_in, mybir.ActivationFunctionType.Exp)

            qkt = [None] * NSB   # [128, SB] bf16: rows 0:64 Q^T, 64:128 K^T
            ktq = [None] * NSB   # [128, SB] bf16: rows 0:64 K^T, 64:128 Q^T
            vext = [None] * (NSB * TPB)  # [128, H+1] bf16: [V | 1]

            def proj_qk(j):
                scol = slice(j * SB, (j + 1) * SB)
                ps_qk = ps.tile([PT, SB], F32, tag="ps")
                for k in range(NDT):
                    nc.tensor.matmul(ps_qk, lhsT=wqk_sb[:, k, :],
                                     rhs=xt_sb[:, k, scol],
                                     start=(k == 0), stop=(k == NDT - 1))
                qkt_j = persist.tile([PT, SB], BF16, tag=f"qkt{j}")
                nc.vector.tensor_scalar_add(qkt_j, ps_qk, bqk_sb)
                qkt[j] = qkt_j
                # swap-selector: ktq = [K^T; Q^T]
                ps_sw = ps.tile([PT, SB], F32, tag="ps")
                nc.tensor.matmul(ps_sw, lhsT=e12_sb, rhs=qkt_j, start=True, stop=True)
                ktq_j = persist.tile([PT, SB], BF16, tag=f"ktq{j}")
                nc.vector.tensor_copy(ktq_j, ps_sw)
                ktq[j] = ktq_j

            def proj_v_packed(j0, j1):
                ps_vv = ps.tile([PT, SB], F32, tag="ps")
                c0 = slice(j0 * SB, (j0 + 1) * SB)
                c1 = slice(j1 * SB, (j1 + 1) * SB)
                for k in range(NDT):
                    nc.tensor.matmul(ps_vv[0:H, :], lhsT=wv_sb[:, k, :],
                                     rhs=xt_sb[:, k, c0],
                                     start=(k == 0), stop=(k == NDT - 1),
                                     skip_group_check=True)
                    nc.tensor.matmul(ps_vv[H:PT, :], lhsT=wv_sb[:, k, :],
                                     rhs=xt_sb[:, k, c1],
                                     start=(k == 0), stop=(k == NDT - 1),
                                     skip_group_check=True)
                vt2 = work.tile([PT, SB], BF16, tag="vt")
                nc.vector.tensor_scalar_add(vt2, ps_vv, bv2_sb)
                for jj, base in ((j0, 0), (j1, H)):
                    for u in range(TPB):
                        tt = jj * TPB + u
                        ps_vt = ps.tile([PT, H], BF16, tag="ps")
                        nc.tensor.transpose(ps_vt,
                                            vt2[base:base + H, u * PT:(u + 1) * PT],
                                            idn2_sb[base:base + H, :])
                        vx = persist.tile([PT, H + 1], BF16, tag=f"vext{tt}")
                        nc.vector.tensor_copy(vx[:, 0:H], ps_vt)
                        nc.vector.memset(vx[:, H:H + 1], 1.0)
                        vext[tt] = vx

            def attn(j):
                ps_o = ps.tile([H + 1, SB], F32, tag="ps")
                last_tt = (j + 1) * TPB - 1
                for p in range(2 * (j + 1)):
                    pair = (2 * p, 2 * p + 1)
                    st = ps_st.tile([PT, 2 * SB], F32, tag="st")
                    # row-packed score matmuls: even tile in rows 0-63,
                    # odd tile in rows 64-127 (concurrent in the PE array)
                    jb0, u0 = divmod(pair[0], TPB)
                    nc.tensor.matmul(st[:, 0:SB],
                                     lhsT=ktq[jb0][0:H, u0 * PT:(u0 + 1) * PT],
                                     rhs=qkt[j][0:H, :],
                                     start=True, stop=True)
                    jb1, u1 = divmod(pair[1], TPB)
                    nc.tensor.matmul(st[:, SB:2 * SB],
                                     lhsT=qkt[jb1][H:PT, u1 * PT:(u1 + 1) * PT],
                                     rhs=ktq[j][H:PT, :],
                                     start=True, stop=True)
                    ex = expp.tile([PT, 2 * SB], BF16, tag="exp")
                    nc.scalar.activation(ex, st, mybir.ActivationFunctionType.Exp,
                                         scale=scale)
                    for q, tt in enumerate(pair):
                        if tt >= j * TPB:  # diagonal tile: zero where t > s
                            nc.gpsimd.affine_select(
                                out=ex[:, q * SB:(q + 1) * SB],
                                in_=ex[:, q * SB:(q + 1) * SB],
                                compare_op=mybir.AluOpType.is_ge,
                                fill=0.0,
                                base=j * SB - tt * PT,
                                pattern=[[1, SB]],
                                channel_multiplier=-1,
                            )
                    for q, tt in enumerate(pair):
                        nc.tensor.matmul(ps_o, lhsT=vext[tt],
                                         rhs=ex[:, q * SB:(q + 1) * SB],
                                         start=(tt == 0), stop=(tt == last_tt))
                ot = otp.tile([H + 1, SB], F32, tag="ot")
                nc.vector.tensor_copy(ot, ps_o)

                ystage = yp.tile([PT, TPB, H], F32, tag="y")
                for u in range(TPB):
                    ps_f = ps.tile([PT, H + 1], F32, tag="ps")
                    nc.tensor.matmul(ps_f, lhsT=ot[:, u * PT:(u + 1) * PT],
                                     rhs=wo_sb, start=True, stop=True)
                    rec = small.tile([PT, 1], F32, tag="rec")
                    nc.vector.reciprocal(rec, ps_f[:, H:H + 1])
                    nc.vector.tensor_scalar_mul(ystage[:, u, :], ps_f[:, 0:H], rec)
                nc.sync.dma_start(
                    out=y_d[j * SB:(j + 1) * SB, :].rearrange("(u p) g -> p u g", p=PT),
                    in_=ystage)

            for jp in range(NSB // 2):
                j0, j1 = 2 * jp, 2 * jp + 1
                proj_qk(j0)
                proj_qk(j1)
                proj_v_packed(j0, j1)
            for j in range(NSB):
                attn(j)
    nc.finalize()  # bacc compile: register alloc + wait splitting (TRN2: <=1 wait/inst)
    return nc


def get_nc():
    if "nc" not in _NC_CACHE:
        _NC_CACHE["nc"] = _build()
    return _NC_CACHE["nc"]


def make_in_maps(x, Wq, bq, Wk, bk, Wv, bv, Wo, bo):
    x = np.asarray(x, np.float32)
    # cst16 = [wqk(8 d-tiles x 128) | wv(8 x 64) | e12(128) | idn2(64)]
    wqk = np.concatenate([np.asarray(Wq).T, np.asarray(Wk).T], axis=1)  # [D, 128]
    wv = np.asarray(Wv).T  # [D, 64]
    cst16 = np.zeros((PT, 1728), np.float32)
    cst16[:, 0:NDT * PT] = wqk.reshape(NDT, PT, PT).transpose(1, 0, 2).reshape(PT, -1)
    cst16[:, NDT * PT:1536] = wv.reshape(NDT, PT, H).transpose(1, 0, 2).reshape(PT, -1)
    e12 = np.zeros((PT, PT), np.float32)  # swap: out rows = [in 64:128; in 0:64]
    e12[H + np.arange(H), np.arange(H)] = 1
    e12[np.arange(H), H + np.arange(H)] = 1
    cst16[:, 1536:1664] = e12
    cst16[0:H, 1664:1728] = np.eye(H)
    cst16[H:PT, 1664:1728] = np.eye(H)
    cst16 = cst16.astype(BF16NP)
    cst32 = np.zeros((PT, H + 3), np.float32)
    cst32[:H, :H] = np.asarray(Wo).T
    cst32[H, :H] = np.asarray(bo)
    cst32[H, H] = 1.0
    cst32[:, H + 1] = np.concatenate([np.asarray(bq), np.asarray(bk)])
    cst32[:, H + 2] = np.concatenate([np.asarray(bv), np.asarray(bv)])
    shared = {"cst16": cst16, "cst32": cst32}
    return [dict(shared, xt=np.ascontiguousarray(x[b].T).astype(BF16NP))
            for b in range(B)]


def kernel(**inputs):
    nc = get_nc()
    in_maps = make_in_maps(**inputs)
    res = run_bass_kernel_spmd(nc, in_maps, list(range(NCORES)))
    return np.stack([res.results[i]["y"] for i in range(NCORES)], axis=0)


# revision 6
# speedup vs baseline: 1.2398x; 1.0023x over previous
"""Trainium2 Bass kernel for a single-head causal attention block (v4).

v4 over v3 (trace: first exp not until 30us, ACT starved; PE cold through
DMA-paced projections; PSUM slot contention serializing phases):
  - QK projections run d-tile-OUTER over an s-block pair with two PSUM
    accumulators, so every arriving x^T d-tile immediately feeds 2 matmuls
  - attention blocks are emitted right after their jp projection group so
    the exp stream starts as early as data allows
  - PSUM pools: 2 accumulator banks + 2 rotating banks + 2x2 score banks
"""

import numpy as np
import ml_dtypes

import concourse.bass as bass
import concourse.bacc as bacc
import concourse.mybir as mybir
from concourse.tile import TileContext
from concourse.bass_utils import run_bass_kernel_spmd

B, S, D, H = 8, 2048, 1024, 64
NCORES = 8
PT = 128           # partition tile
SB = 512           # s-block width (moving-operand width)
NSB = S // SB      # 4
NDT = D // PT      # 8 d-tiles
TPB = SB // PT     # 4 t-tiles per s-block
BF16 = mybir.dt.bfloat16
F32 = mybir.dt.float32
BF16NP = ml_dtypes.bfloat16

_NC_CACHE = {}


def _build():
    nc = bacc.Bacc()
    xt_d = nc.dram_tensor("xt", [D, S], BF16, kind="ExternalInput")
    # packed consts: cst16 = [wqk(8x128) | wv(8x64) | e12(128) | idn2(64)]
    cst16_d = nc.dram_tensor("cst16", [PT, 1728], BF16, kind="ExternalInput")
    # cst32 = [wo(65, zero-padded to 128 rows) | bqk(1) | bv2(1)]
    cst32_d = nc.dram_tensor("cst32", [PT, H + 3], F32, kind="ExternalInput")
    y_d = nc.dram_tensor("y", [S, H], F32, kind="ExternalOutput")

    scale = 1.0 / float(np.sqrt(H))

    with TileContext(nc) as tc:
        with (
            tc.tile_pool(name="consts", bufs=1) as consts,
            tc.tile_pool(name="persist", bufs=1) as persist,
            tc.tile_pool(name="work", bufs=2) as work,
            tc.tile_pool(name="expp", bufs=3) as expp,
            tc.tile_pool(name="otp", bufs=2) as otp,
            tc.tile_pool(name="yp", bufs=2) as yp,
            tc.tile_pool(name="small", bufs=4) as small,
            tc.tile_pool(name="acc", bufs=2, space="PSUM") as accp,
            tc.tile_pool(name="ps", bufs=2, space="PSUM") as ps,
            tc.tile_pool(name="ps_st", bufs=2, space="PSUM") as ps_st,
        ):
            # ---- consts in two packed DMAs, then 8 big x^T DMAs
            cst16 = consts.tile([PT, 1728], BF16, tag="cst16")
            nc.sync.dma_start(out=cst16, in_=cst16_d[:])
            cst32 = consts.tile([PT, H + 3], F32, tag="cst32")
            nc.sync.dma_start(out=cst32, in_=cst32_d[:])
            wqk_sb = cst16[:, 0:NDT * PT].rearrange("p (k m) -> p k m", k=NDT)
            wv_sb = cst16[:, NDT * PT:NDT * PT + NDT * H].rearrange(
                "p (k m) -> p k m", k=NDT)
            e12_sb = cst16[:, 1536:1536 + PT]
            idn2_sb = cst16[:, 1664:1664 + H]
            wo_sb = cst32[0:H + 1, 0:H + 1]
            bqk_sb = cst32[:, H + 1:H + 2]
            bv2_sb = cst32[:, H + 2:H + 3]

            xt_sb = persist.tile([PT, NDT, S], BF16, tag="xt")
            for k in range(NDT):
                nc.sync.dma_start(out=xt_sb[:, k, :],
                                  in_=xt_d[k * PT:(k + 1) * PT, :])

            # warm the ACT exp table while DMA streams in
            warm_in = small.tile([PT, 1], F32, tag="warm_in")
            nc.vector.memset(warm_in, 0.0)
            warm_out = small.tile([PT, 1], F32, tag="warm_out")
            nc.scalar.activation(warm_out, warm_in, mybir.ActivationFunctionType.Exp)

            qkt = [None] * NSB   # [128, SB] bf16: rows 0:64 Q^T, 64:128 K^T
            ktq = [None] * NSB   # [128, SB] bf16: rows 0:64 K^T, 64:128 Q^T
            vext = [None] * (NSB * TPB)  # [128, H+1] bf16: [V | 1]

            def proj_pair(j0, j1):
                # d-outer: each arriving d-tile feeds both s-blocks' QK matmul
                c0 = slice(j0 * SB, (j0 + 1) * SB)
                c1 = slice(j1 * SB, (j1 + 1) * SB)
                a0 = accp.tile([PT, SB], F32, tag="acc")
                a1 = accp.tile([PT, SB], F32, tag="acc")
                for k in range(NDT):
                    nc.tensor.matmul(a0, lhsT=wqk_sb[:, k, :], rhs=xt_sb[:, k, c0],
                                     start=(k == 0), stop=(k == NDT - 1))
                    nc.tensor.matmul(a1, lhsT=wqk_sb[:, k, :], rhs=xt_sb[:, k, c1],
                                     start=(k == 0), stop=(k == NDT - 1))
                for j, a in ((j0, a0), (j1, a1)):
                    qkt_j = persist.tile([PT, SB], BF16, tag=f"qkt{j}")
                    nc.vector.tensor_scalar_add(qkt_j, a, bqk_sb)
                    qkt[j] = qkt_j
                    # swap-selector: ktq = [K^T; Q^T]
                    ps_sw = ps.tile([PT, SB], F32, tag="ps")
                    nc.tensor.matmul(ps_sw, lhsT=e12_sb, rhs=qkt_j,
                                     start=True, stop=True)
                    ktq_j = persist.tile([PT, SB], BF16, tag=f"ktq{j}")
                    nc.vector.tensor_copy(ktq_j, ps_sw)
                    ktq[j] = ktq_j
                # col-packed V^T for both s-blocks (x^T is resident by now)
                vv = accp.tile([PT, SB], F32, tag="acc")
                for k in range(NDT):
                    nc.tensor.matmul(vv[0:H, :], lhsT=wv_sb[:, k, :],
                                     rhs=xt_sb[:, k, c0],
                                     start=(k == 0), stop=(k == NDT - 1),
                                     skip_group_check=True)
                    nc.tensor.matmul(vv[H:PT, :], lhsT=wv_sb[:, k, :],
                                     rhs=xt_sb[:, k, c1],
                                     start=(k == 0), stop=(k == NDT - 1),
                                     skip_group_check=True)
                vt2 = work.tile([PT, SB], BF16, tag="vt")
                nc.vector.tensor_scalar_add(vt2, vv, bv2_sb)
                for jj, base in ((j0, 0), (j1, H)):
                    for u in range(TPB):
                        tt = jj * TPB + u
                        ps_vt = ps.tile([PT, H], BF16, tag="ps")
                        nc.tensor.transpose(ps_vt,
                                            vt2[base:base + H, u * PT:(u + 1) * PT],
                                            idn2_sb[base:base + H, :])
                        vx = persist.tile([PT, H + 1], BF16, tag=f"vext{tt}")
                        nc.vector.tensor_copy(vx[:, 0:H], ps_vt)
                        nc.vector.memset(vx[:, H:H + 1], 1.0)
                        vext[tt] = vx

            def attn(j):
                ps_o = ps.tile([H + 1, SB], F32, tag="ps")
                last_tt = (j + 1) * TPB - 1
                for p in range(2 * (j + 1)):
                    pair = (2 * p, 2 * p + 1)
                    st = ps_st.tile([PT, 2 * SB], F32, tag="st")
                    # row-packed score matmuls: even tile in rows 0-63,
                    # odd tile in rows 64-127 (concurrent in the PE array)
                    jb0, u0 = divmod(pair[0], TPB)
                    nc.tensor.matmul(st[:, 0:SB],
                                     lhsT=ktq[jb0][0:H, u0 * PT:(u0 + 1) * PT],
                                     rhs=qkt[j][0:H, :],
                                     start=True, stop=True)
                    jb1, u1 = divmod(pair[1], TPB)
                    nc.tensor.matmul(st[:, SB:2 * SB],
                                     lhsT=qkt[jb1][H:PT, u1 * PT:(u1 + 1) * PT],
                                     rhs=ktq[j][H:PT, :],
                                     start=True, stop=True)
                    ex = expp.tile([PT, 2 * SB], BF16, tag="exp")
                    nc.scalar.activation(ex, st, mybir.ActivationFunctionType.Exp,
                                         scale=scale)
                    for q, tt in enumerate(pair):
                        if tt >= j * TPB:  # diagonal tile: zero where t > s
                            nc.gpsimd.affine_select(
                                out=ex[:, q * SB:(q + 1) * SB],
                                in_=ex[:, q * SB:(q + 1) * SB],
                                compare_op=mybir.AluOpType.is_ge,
                                fill=0.0,
                                base=j * SB - tt * PT,
                                pattern=[[1, SB]],
                                channel_multiplier=-1,
                            )
                    for q, tt in enumerate(pair):
                        nc.tensor.matmul(ps_o, lhsT=vext[tt],
                                         rhs=ex[:, q * SB:(q + 1) * SB],
                                         start=(tt == 0), stop=(tt == last_tt))
                ot = otp.tile([H + 1, SB], F32, tag="ot")
                nc.vector.tensor_copy(ot, ps_o)

                ystage = yp.tile([PT, TPB, H], F32, tag="y")
                for u in range(TPB):
                    ps_f = ps.tile([PT, H + 1], F32, tag="ps")
                    nc.tensor.matmul(ps_f, lhsT=ot[:, u * PT:(u + 1) * PT],
                                     rhs=wo_sb, start=True, stop=True)
                    rec = small.tile([PT, 1], F32, tag="rec")
                    nc.vector.reciprocal(rec, ps_f[:, H:H + 1])
                    nc.vector.tensor_scalar_mul(ystage[:, u, :], ps_f[:, 0:H], rec)
                nc.sync.dma_start(
                    out=y_d[j * SB:(j + 1) * SB, :].rearrange("(u p) g -> p u g", p=PT),
                    in_=ystage)

            for jp in range(NSB // 2):
                j0, j1 = 2 * jp, 2 * jp + 1
                proj_pair(j0, j1)
                attn(j0)
                attn(j1)
    nc.finalize()  # bacc compile: register alloc + wait splitting (TRN2: <=1 wait/inst)
    return nc


def get_nc():
    if "nc" not in _NC_CACHE:
        _NC_CACHE["nc"] = _build()
    return _NC_CACHE["nc"]


def make_in_maps(x, Wq, bq, Wk, bk, Wv, bv, Wo, bo):
    x = np.asarray(x, np.float32)
    # cst16 = [wqk(8 d-tiles x 128) | wv(8 x 64) | e12(128) | idn2(64)]
    wqk = np.concatenate([np.asarray(Wq).T, np.asarray(Wk).T], axis=1)  # [D, 128]
    wv = np.asarray(Wv).T  # [D, 64]
    cst16 = np.zeros((PT, 1728), np.float32)
    cst16[:, 0:NDT * PT] = wqk.reshape(NDT, PT, PT).transpose(1, 0, 2).reshape(PT, -1)
    cst16[:, NDT * PT:1536] = wv.reshape(NDT, PT, H).transpose(1, 0, 2).reshape(PT, -1)
    e12 = np.zeros((PT, PT), np.float32)  # swap: out rows = [in 64:128; in 0:64]
    e12[H + np.arange(H), np.arange(H)] = 1
    e12[np.arange(H), H + np.arange(H)] = 1
    cst16[:, 1536:1664] = e12
    cst16[0:H, 1664:1728] = np.eye(H)
    cst16[H:PT, 1664:1728] = np.eye(H)
    cst16 = cst16.astype(BF16NP)
    cst32 = np.zeros((PT, H + 3), np.float32)
    cst32[:H, :H] = np.asarray(Wo).T
    cst32[H, :H] = np.asarray(bo)
    cst32[H, H] = 1.0
    cst32[:, H + 1] = np.concatenate([np.asarray(bq), np.asarray(bk)])
    cst32[:, H + 2] = np.concatenate([np.asarray(bv), np.asarray(bv)])
    shared = {"cst16": cst16, "cst32": cst32}
    return [dict(shared, xt=np.ascontiguousarray(x[b].T).astype(BF16NP))
            for b in range(B)]


def kernel(**inputs):
    nc = get_nc()
    in_maps = make_in_maps(**inputs)
    res = run_bass_kernel_spmd(nc, in_maps, list(range(NCORES)))
    return np.stack([res.results[i]["y"] for i in range(NCORES)], axis=0)
